# revision 36
# baseline (speedup 1.0000x reference)
"""Trainium2 Bass kernel for the GuidedAtt mesh-attention module.

Strategy: data-parallel over batch B=8 across 8 NeuronCores (one batch item
per core).  All tensors ride in bf16 (tolerance is 2e-2; this lands ~4e-3).

Layout: vertices are laid out v = c*128 + p.  Phase B runs the GEMMs with
vertex tiles of 128 on PSUM partitions, channels on the free axis.  Per tile:

  hp  = dft^T @ W1 + darea-row @ w1l (fp8 K=1)        PE (hp/gp split pools)
  h   = tanh(hp)                                      Act (batched x4)
  sa  = sigmoid(reduce(h * w3) + b3)   Pool mult + tree-add, DVE reduce, Act
  out = max(gp, 0) * sa  with gp = dft^T @ Wp         DVE stt (1 in 6 quads
                                                      on Act via Relu+scale)

GPSIMD cannot touch PSUM and TensorScalarPtr is DVE-only on real silicon, so
the PSUM-evicting gating rides DVE/Act while the SBUF-side sa chain rides
Pool.  The rank-1 darea stationary must live on a single SBUF partition and
stationaries may only start at partitions 0/32/64, so darea columns are
XBAR-transposed, cast to fp8, bounced through a DRAM scratch, and loaded as
three q-group rows (107 tiles each) at partitions 0/32/64; a junk second
partition row in each load keeps the big contiguous dim last (full DMA rate).

Phase A (vertex areas) consumes host-composed per-slot corner coordinates
(zero device gathers), normalizes/crosses in bf16 split across DVE and Pool,
and produces darea in [128, 321] column-major form.  Weights/constants ride
in one packed [128, 1602] bf16 blob (single DMA).  Output is written as
out^T [NPAD, 256] bf16; the host transposes back.
"""

import sys
sys.path.insert(0, "/opt/trn_rl_repo")

import numpy as np
import ml_dtypes

import concourse.bass as bass
import concourse.mybir as mybir
import concourse.tile as tile
from concourse import bacc
from concourse.bass_utils import run_bass_kernel_spmd

B, N_VERTS, N_FACES = 8, 40962, 81920
IN_CH, HD_CH, PROJ_CH = 128, 256, 256

P = 128
VC = 321                     # vertex columns (v = c*128 + p), NPAD = 41088
NPAD = P * VC
NT = VC                      # 321 vertex tiles of 128
SC = VC * 6                  # 1926 slot columns
SCX = SC + 1                 # +1 leading face-0 column
QN = 107                     # tiles per q-group (dr4 partition homes 0/32/64)
DRW = QN * 128               # dr4 per-partition darea-row bytes (fp8)
F32 = mybir.dt.float32
BF16 = mybir.dt.bfloat16
FP8 = mybir.dt.float8e4
BF = ml_dtypes.bfloat16
E4 = ml_dtypes.float8_e4m3

_CACHE = {}


def build(b1nz=False):
    key = f"v4_b1nz{b1nz}"
    if key not in _CACHE:
        _CACHE[key] = _build_program(b1nz)
    return _CACHE[key]


def _build_program(b1nz=False):
    nc = bacc.Bacc("TRN2", target_bir_lowering=False, debug=False, num_devices=8)

    slots_d = nc.declare_dram_parameter("slots", (P, 9, SCX), BF16, isOutput=False)
    dfeat_d = nc.declare_dram_parameter("dfeat", (P, NPAD), BF16, isOutput=False)
    wblob_d = nc.declare_dram_parameter("wblob", (P, 1602), BF16, isOutput=False)
    drs_d = nc.declare_dram_parameter("drscratch", (4 * DRW,), FP8, isOutput=False)
    out_d = nc.declare_dram_parameter("out", (NPAD, PROJ_CH), BF16, isOutput=True)

    Tanh = mybir.ActivationFunctionType.Tanh
    Sigmoid = mybir.ActivationFunctionType.Sigmoid
    Sqrt = mybir.ActivationFunctionType.Sqrt
    Relu = mybir.ActivationFunctionType.Relu
    add_op = mybir.AluOpType.add
    sub_op = mybir.AluOpType.subtract
    mul_op = mybir.AluOpType.mult
    max_op = mybir.AluOpType.max

    with tile.TileContext(nc) as tc:
        with tc.tile_pool(name="persist", bufs=1) as persist, \
             tc.tile_pool(name="aio", bufs=2) as aio, \
             tc.tile_pool(name="awk", bufs=2) as awk, \
             tc.tile_pool(name="gio", bufs=6) as gio, \
             tc.tile_pool(name="hwk", bufs=3) as hwk, \
             tc.tile_pool(name="oio", bufs=3) as oio, \
             tc.tile_pool(name="psh", bufs=2, space="PSUM") as psh, \
             tc.tile_pool(name="psg", bufs=2, space="PSUM") as psg:

            W = 81 * 6 + 2            # widest chunk work width (chunk cw<=81)
            st01 = aio.tile([P, 9, W], BF16, tag="slotin", name="st01")
            nc.sync.dma_start(out=st01[:, :, 0:481], in_=slots_d[:, :, 0:481])
            wblob_s = persist.tile([P, 1602], BF16, tag="wblob")
            w1l_s = persist.tile([P, HD_CH], FP8, tag="w1l")
            ones_s = persist.tile([P, P], BF16, tag="ones")
            b3_s = persist.tile([P, 1], F32, tag="b3")
            wblob = wblob_s[:]
            w1_s = wblob[:, 0:256]
            w3_s = wblob[:, 256:512]
            wp_s = wblob[:, 512:768]
            area0_v = wblob[:, 768:1089]
            b1_v = wblob[:, 1346:1602]
            nc.sync.dma_start(out=wblob, in_=wblob_d[:])
            nc.vector.tensor_copy(out=w1l_s[0:65, :], in_=wblob[0:65, 1090:1346])
            nc.vector.tensor_copy(out=b3_s[:], in_=wblob[:, 1089:1090])
            for _q in range(3):
                nc.vector.memset(ones_s[_q * 32:_q * 32 + 1, :], 1.0)
            warm_s = persist.tile([1, 1], BF16, tag="warm")
            nc.scalar.activation(warm_s[:], ones_s[0:1, 0:1], Sqrt)

            varea = persist.tile([P, VC], F32, tag="varea")
            # darea column-major, padded to 384 for 32-wide XBAR transposes
            darea_pm = persist.tile([P, 384], BF16, tag="dareapm")
            t_all = persist.tile([P, 4, P], BF16, tag="tall")
            t_all8 = persist.tile([P, 4, P], FP8, tag="tall8")
            # dr4: per-q-group darea rows at partitions 0/32/64/96
            dr4 = persist.tile([P, DRW], FP8, tag="dr4")
            nc.vector.memset(darea_pm[:], 0.0)

            # ================= Phase A: vertex areas =================

            def chain_n2(eng, st, n2a, lo, hi, t2):
                wdt = hi - lo
                for pt in range(3):
                    x = st[:, 3 * pt + 0, lo:hi]
                    y = st[:, 3 * pt + 1, lo:hi]
                    z = st[:, 3 * pt + 2, lo:hi]
                    n2 = n2a[:, pt, lo:hi]
                    yield eng.tensor_tensor(out=n2, in0=x, in1=x, op=mul_op)
                    yield eng.tensor_tensor(out=t2[:, :wdt], in0=y, in1=y, op=mul_op)
                    yield eng.tensor_tensor(out=n2, in0=n2, in1=t2[:, :wdt], op=add_op)
                    yield eng.tensor_tensor(out=t2[:, :wdt], in0=z, in1=z, op=mul_op)
                    yield eng.tensor_tensor(out=n2, in0=n2, in1=t2[:, :wdt], op=add_op)

            def chain_rest(eng, st, u, e, rn, fa, lo, hi, cr, t3):
                wdt = hi - lo
                for pt in range(3):
                    for cc in range(3):
                        yield eng.tensor_tensor(out=u[:, 3 * pt + cc, lo:hi],
                                          in0=st[:, 3 * pt + cc, lo:hi],
                                          in1=rn[:, pt, lo:hi], op=mul_op)
                for cc in range(3):
                    yield eng.tensor_tensor(out=e[:, cc, lo:hi], in0=u[:, cc, lo:hi],
                                      in1=u[:, 3 + cc, lo:hi], op=sub_op)
                    yield eng.tensor_tensor(out=e[:, 3 + cc, lo:hi],
                                      in0=u[:, cc, lo:hi],
                                      in1=u[:, 6 + cc, lo:hi], op=sub_op)
                fa2 = fa  # accumulate |cross|^2 in fa's slot, sqrt in place
                for cc in range(3):
                    i0, i1 = (cc + 1) % 3, (cc + 2) % 3
                    yield eng.tensor_tensor(out=cr[:, :wdt], in0=e[:, i0, lo:hi],
                                      in1=e[:, 3 + i1, lo:hi], op=mul_op)
                    yield eng.tensor_tensor(out=t3[:, :wdt], in0=e[:, i1, lo:hi],
                                      in1=e[:, 3 + i0, lo:hi], op=mul_op)
                    yield eng.tensor_tensor(out=cr[:, :wdt], in0=cr[:, :wdt],
                                      in1=t3[:, :wdt], op=sub_op)
                    if cc == 0:
                        yield eng.tensor_tensor(out=fa2[:, lo:hi], in0=cr[:, :wdt],
                                          in1=cr[:, :wdt], op=mul_op)
                    else:
                        yield eng.tensor_tensor(out=t3[:, :wdt], in0=cr[:, :wdt],
                                          in1=cr[:, :wdt], op=mul_op)
                        yield eng.tensor_tensor(out=fa2[:, lo:hi], in0=fa2[:, lo:hi],
                                          in1=t3[:, :wdt], op=add_op)

            def load_slots(c0, cw, eng=None):
                off = 1 if c0 == 0 else 0
                swx = cw * 6 + off
                s0 = 0 if c0 == 0 else 1 + c0 * 6
                st = aio.tile([P, 9, W], BF16, tag="slotin", name="st")
                (eng or nc.sync).dma_start(out=st[:, :, :swx],
                                           in_=slots_d[:, :, s0:s0 + swx])
                return st

            def emit_chunk(c0, cw, st=None, sto=0):
                # generator; c0/cw in vertex columns.  st: preloaded slot
                # tile; sto: column offset of this chunk within st.
                sw = cw * 6
                off = 1 if c0 == 0 else 0
                swx = sw + off
                if st is None:
                    st = load_slots(c0, cw)
                if sto:
                    st = st[:, :, sto:]

                n2a = awk.tile([P, 3, W], BF16, tag="n2a")
                rn = awk.tile([P, 3, W], BF16, tag="rn")
                u = awk.tile([P, 9, W], BF16, tag="u")
                e = awk.tile([P, 6, W], BF16, tag="e")
                fa = awk.tile([P, W], BF16, tag="fa")
                t2d = awk.tile([P, 640], BF16, tag="t2d")
                crd = awk.tile([P, 640], BF16, tag="crd")
                t3d = awk.tile([P, 640], BF16, tag="t3d")
                t2p = awk.tile([P, 640], BF16, tag="t2p")
                crp = awk.tile([P, 640], BF16, tag="crp")
                t3p = awk.tile([P, 640], BF16, tag="t3p")

                # independent column-halves split DVE : Pool (DVE ~3.2x faster
                # per column but also carries the phase-B sa work)
                cwd = cw * 95 // 128
                mid = off + cwd * 6

                def rr(ga, gb):
                    while True:
                        done = 0
                        for g in (ga, gb):
                            try:
                                next(g)
                                yield
                            except StopIteration:
                                done += 1
                        if done == 2:
                            return

                yield from rr(chain_n2(nc.vector, st, n2a, 0, mid, t2d),
                              chain_n2(nc.gpsimd, st, n2a, mid, swx, t2p))
                nc.scalar.activation(u[:, 0:3, :swx], n2a[:, :, :swx], Sqrt)
                with nc.allow_low_precision(reason="unit-vec scale bf16"):
                    nc.vector.reciprocal(out=rn[:, :, :swx],
                                         in_=u[:, 0:3, :swx])
                yield
                yield from rr(chain_rest(nc.vector, st, u, e, rn, fa, 0, mid,
                                         crd, t3d),
                              chain_rest(nc.gpsimd, st, u, e, rn, fa, mid, swx,
                                         crp, t3p))
                nc.scalar.activation(fa[:, :swx], fa[:, :swx], Sqrt)
                yield

                # 6:1 reduction into varea columns
                nc.vector.tensor_reduce(
                    out=varea[:, c0:c0 + cw],
                    in_=fa[:, off:off + sw].rearrange("p (v k) -> p v k", k=6),
                    axis=mybir.AxisListType.X, op=add_op)
                if c0 == 0:
                    # first 12 vertices live on partitions 0..11, column 0
                    nc.vector.tensor_tensor(
                        out=varea[0:12, 0:1], in0=varea[0:12, 0:1],
                        in1=fa[0:12, 0:1], op=sub_op)
                # darea = varea/6 - area0
                nc.vector.scalar_tensor_tensor(
                    out=darea_pm[:, c0:c0 + cw], in0=varea[:, c0:c0 + cw],
                    scalar=1.0 / 6.0, in1=area0_v[:, c0:c0 + cw],
                    op0=mul_op, op1=sub_op)

            def emit_xbar(tb):
                # full-block XBAR transpose of darea column block tb + fp8 cast
                nc.sync.dma_start(out=t_all[:, tb, :],
                                  in_=darea_pm[:, tb * P:(tb + 1) * P],
                                  transpose=True)
                nc.vector.tensor_copy(out=t_all8[:, tb, :],
                                      in_=t_all[:, tb, :])

            def emit_store(q, tb, clo, chi, j0):
                # t_all8 rows [clo,chi) of block tb -> DRAM scratch rows j0..
                rw = chi - clo
                nc.sync.dma_start(
                    out=drs_d[q * DRW + j0 * P:q * DRW + (j0 + rw) * P].rearrange(
                        "(j v) -> j v", v=P),
                    in_=t_all8[clo:chi, tb, :])

            def emit_load(q, r0, rw):
                # DRAM scratch -> dr4 row at partition q*32.  A second junk
                # partition row keeps the big contiguous dim last in the AP
                # (avoids the small-elem DMA penalty).
                nc.sync.dma_start(
                    out=dr4[q * 32:q * 32 + 2, r0 * P:(r0 + rw) * P],
                    in_=drs_d[q * DRW + r0 * P:q * DRW + (rw * P) * 2 + r0 * P]
                        .rearrange("(two n) -> two n", two=2))

            df_cache = {}

            def load_df(g):
                # one 8-tile group (1024 cols) per load
                if g in df_cache or g > 40:
                    return df_cache.get(g)
                df_t = gio.tile([P, 1024], BF16, tag="dfeat", name="df_t")
                c0 = g * 1024
                nc.sync.dma_start(out=df_t[:, :min(1024, NPAD - c0)],
                                  in_=dfeat_d[:, c0:min(c0 + 1024, NPAD)])
                df_cache[g] = df_t
                return df_t

            # ============ Phase B: fused GEMM pipeline ============
            pair_ot = {}

            def quad_produce(g):
                """4 tiles t0..t0+3: matmuls, tanh, sa mult + tree reduce."""
                t0 = 4 * g
                jn = min(4, NT - t0)
                pi = g // 2
                df_t = load_df(pi)
                load_df(pi + 3)            # prefetch three pairs ahead
                dfo = (g % 2) * 512
                hp = psh.tile([P, 4, HD_CH], F32, tag="hp", name="hp")
                gp = psg.tile([P, 4, PROJ_CH], F32, tag="gp", name="gp")
                for j in range(jn):
                    t = t0 + j
                    q = min(t // QN, 2)
                    r = t - q * QN
                    dft = df_t[:, dfo + j * P:dfo + (j + 1) * P]
                    nc.tensor.matmul(hp[:, j, :], dft, w1_s,
                                     start=True, stop=False)
                    seg = dr4[q * 32:q * 32 + 1, r * P:(r + 1) * P]
                    w1l_q = w1l_s[q * 32:q * 32 + 1, :]
                    if b1nz:
                        nc.tensor.matmul(hp[:, j, :], seg, w1l_q,
                                         start=False, stop=False)
                        nc.tensor.matmul(hp[:, j, :],
                                         ones_s[q * 32:q * 32 + 1, :],
                                         b1_v[q * 32:q * 32 + 1, :],
                                         start=False, stop=True)
                    else:
                        nc.tensor.matmul(hp[:, j, :], seg, w1l_q,
                                         start=False, stop=True)
                    nc.tensor.matmul(gp[:, j, :], dft, wp_s,
                                     start=True, stop=True)
                h_sb = hwk.tile([P, 4, HD_CH], BF16, tag="h", name="h_sb")
                wh = hwk.tile([P, 4, HD_CH], BF16, tag="wh", name="wh")
                a1 = hwk.tile([P, 4, 128], BF16, tag="a1", name="a1")
                a2 = hwk.tile([P, 4, 64], BF16, tag="a2", name="a2")
                sa_pre = hwk.tile([P, 4, 1], F32, tag="sapre", name="sa_pre")
                nc.scalar.activation(h_sb[:, :jn, :], hp[:, :jn, :], Tanh)
                nc.gpsimd.tensor_tensor(
                    out=wh[:, :jn, :], in0=h_sb[:, :jn, :],
                    in1=w3_s.rearrange("p (one c) -> p one c", one=1)
                        .to_broadcast([P, jn, HD_CH]),
                    op=mul_op)
                with nc.allow_low_precision(reason="sa tree-reduce bf16"):
                    nc.gpsimd.tensor_tensor(
                        out=a1[:, :jn, :], in0=wh[:, :jn, 0:128],
                        in1=wh[:, :jn, 128:256], op=add_op)
                    nc.gpsimd.tensor_tensor(
                        out=a2[:, :jn, :], in0=a1[:, :jn, 0:64],
                        in1=a1[:, :jn, 64:128], op=add_op)
                nc.vector.tensor_reduce(
                    out=sa_pre[:, :jn, 0], in_=a2[:, :jn, :],
                    axis=mybir.AxisListType.X, op=add_op)
                return (g, jn, gp, sa_pre)

            def quad_consume(st):
                g, jn, gp, sa_pre = st
                pi = g // 2
                oto = (g % 2) * 4
                if pi not in pair_ot:
                    pair_ot[pi] = oio.tile([P, 8, PROJ_CH], BF16, tag="out",
                                           name="ot")
                ot = pair_ot[pi]
                sa_sig = hwk.tile([P, 4, 1], F32, tag="sasig", name="sa_sig")
                nc.scalar.activation(sa_sig[:, :jn, :], sa_pre[:, :jn, :],
                                     Sigmoid, bias=b3_s[:])
                if g % 6 == 5:
                    # Act path: relu(gp * sa) == sa * relu(gp) since sa > 0;
                    # per-partition scale forces one op per tile
                    for j in range(jn):
                        nc.scalar.activation(ot[:, oto + j, :], gp[:, j, :],
                                             Relu, scale=sa_sig[:, j, :])
                else:
                    nc.vector.scalar_tensor_tensor(
                        out=ot[:, oto:oto + jn, :], in0=gp[:, :jn, :],
                        scalar=0.0,
                        in1=sa_sig[:, :jn, :].to_broadcast([P, jn, PROJ_CH]),
                        op0=max_op, op1=mul_op)
                if g % 2 == 1 or g == 80:
                    t0 = 8 * pi
                    tn = min(8, NT - t0)
                    nc.sync.dma_start(
                        out=out_d[t0 * P:(t0 + tn) * P, :].rearrange(
                            "(g p) c -> p g c", p=P),
                        in_=ot[:, :tn, :])
                    del pair_ot[pi]

            defer = []

            def emit_quad(g):
                defer.append(quad_produce(g))
                if len(defer) > 1:
                    quad_consume(defer.pop(0))

            def flush_quads():
                while defer:
                    quad_consume(defer.pop(0))

            pending = []

            def start_chunk(c0, cw, st=None, sto=0):
                pending.append(emit_chunk(c0, cw, st, sto))

            def drive(n):
                while pending and n > 0:
                    try:
                        for _ in range(n):
                            next(pending[0])
                            n -= 1
                    except StopIteration:
                        pending.pop(0)

            def drain():
                while pending:
                    for _ in pending.pop(0):
                        pass

            # ---- interleaved emission ----
            # phase-A chunks 16/64/80/80/81 cols; dr homes per 107 tiles
            load_df(0), load_df(1), load_df(2)
            start_chunk(0, 16, st01)
            drain()
            emit_xbar(0)
            emit_store(0, 0, 0, 16, 0)
            emit_load(0, 0, 16)
            start_chunk(16, 64, st01, 97)
            for g in range(4):                 # tiles 0..15
                emit_quad(g)
                drive(16)
            drain()
            emit_xbar(0)
            emit_store(0, 0, 16, 80, 16)
            emit_load(0, 16, 64)
            for g in range(4, 10):             # tiles 16..39
                emit_quad(g)
                drive(8)
            start_chunk(80, 80)
            for g in range(10, 20):            # tiles 40..79
                emit_quad(g)
                drive(10)
            drain()
            emit_xbar(0)
            emit_xbar(1)
            emit_store(0, 0, 80, 107, 80)
            emit_load(0, 80, 27)               # tiles 80..106
            emit_store(1, 0, 107, 128, 0)
            emit_store(1, 1, 0, 32, 21)
            emit_load(1, 0, 53)                # tiles 107..159
            start_chunk(160, 80)
            for g in range(20, 40):            # tiles 80..159
                emit_quad(g)
                drive(5)
            drain()
            emit_xbar(1)
            emit_store(1, 1, 32, 86, 53)
            emit_load(1, 53, 54)               # tiles 160..213
            emit_store(2, 1, 86, 112, 0)
            emit_load(2, 0, 26)                # tiles 214..239
            start_chunk(240, 81)
            for g in range(40, 60):            # tiles 160..239
                emit_quad(g)
                drive(5)
            drain()
            emit_xbar(1)
            emit_xbar(2)
            emit_store(2, 1, 112, 128, 26)
            emit_store(2, 2, 0, 65, 42)
            emit_load(2, 26, 81)               # tiles 240..320
            for g in range(60, 81):            # tiles 240..320
                emit_quad(g)
            flush_quads()

    nc.compile()
    return nc


def _prep_host(vnew, dfeat, f0, vfids, area0, W1, b1, W3, b3, Wp, bp):
    """Host does only index composition, layout, and dtype casts."""
    if np.any(np.asarray(bp)):
        raise NotImplementedError(
            "nonzero bp needs the generic gate path (relu(sa*g4+bp))")
    pad = NPAD - N_VERTS
    corner = np.asarray(f0)[np.asarray(vfids)]          # [N, 6, 3] vertex ids
    corner_p = np.concatenate(
        [corner, np.zeros((pad, 6, 3), np.int32)], axis=0)

    area0_pm = np.concatenate(
        [np.asarray(area0).reshape(-1).astype(np.float32),
         np.zeros(pad, np.float32)]).reshape(VC, P).T.astype(BF)

    W1 = np.asarray(W1, np.float32)
    wblob = np.zeros((P, 1602), BF)
    wblob[:, 0:256] = np.ascontiguousarray(W1[:IN_CH]).astype(BF)
    wblob[:, 256:512] = np.asarray(W3, np.float32).reshape(1, HD_CH).astype(BF)
    wblob[:, 512:768] = np.asarray(Wp, np.float32).astype(BF)
    wblob[:, 768:1089] = area0_pm
    wblob[:, 1089] = np.float32(np.asarray(b3).reshape(())).astype(BF)
    wblob[0:65:32, 1090:1346] = W1[IN_CH].astype(BF)
    wblob[0:65:32, 1346:1602] = np.asarray(b1, np.float32).astype(BF)
    drscratch = np.zeros(4 * DRW, E4)

    in_maps = []
    for bidx in range(B):
        v = np.asarray(vnew[bidx], np.float32)
        vpad = np.concatenate(
            [v, np.tile(np.array([[1.0, 0, 0]], np.float32), (pad, 1))], axis=0)
        cc = vpad[corner_p]                              # [NPAD, 6, 3, 3]
        cc = cc.reshape(VC, P, 6, 9)                     # (c, p, k, plane)
        slots = np.empty((P, 9, SCX), BF)
        slots[:, :, 1:] = cc.transpose(1, 3, 0, 2).reshape(P, 9, SC).astype(BF)
        slots[:, :, 0] = vpad[np.asarray(f0)[0]].reshape(9)[None, :].astype(BF)
        dfp = np.zeros((P, NPAD), BF)
        dfp[:, :N_VERTS] = np.asarray(dfeat[bidx], np.float32).astype(BF)
        in_maps.append({
            "slots": slots, "dfeat": dfp, "wblob": wblob,
            "drscratch": drscratch,
        })
    return in_maps


def _post_host(results):
    outs = []
    for res in results:
        o = np.asarray(res["out"])[:N_VERTS]             # [N, 256] bf16
        outs.append(np.ascontiguousarray(o.T).astype(np.float32))
    return outs


def kernel(**inputs):
    nc = build(b1nz=bool(np.any(np.asarray(inputs["b1"]))))
    in_maps = _prep_host(**inputs)
    res = run_bass_kernel_spmd(nc, in_maps, core_ids=list(range(B)))
    return np.stack(_post_host([res.results[i] for i in range(B)]), axis=0)


if __name__ == "__main__":
    print("standalone build test")
    build()
    print("build ok")
